# revision 12
# baseline (speedup 1.0000x reference)
"""Trainium2 Bass kernel for nn_CrossLayer (DCN-style cross stack).

Reference semantics (B=16384, D=1024, L=8):
    out_0 = x
    s_i = einsum('bd,d->b', out_i, W[i])
    out_{i+1} = x * s_i[:, None] + b[i] + x

Algebraic collapse: out_{i+1} = x * rho_{i+1} + b[i] with
    rho_1 = u_0 + 1,   rho_{l+1} = rho_l * u_l + c_l
    u_l[r] = <x[r, :], W[l]>          (U = x @ W.T, [B, L])
    c_l = <b[l-1], W[l]> + 1          (weights-only scalars)
    out = x * rho_8[:, None] + b[L-1]

Device work: U = x @ W.T (PE transposes + W-stationary matmuls), a 7-step
per-row scan, one fused scale+bias pass. x read once, out written once ->
memory-roofline bound. All arithmetic fp32 (exact vs reference).

Sharding: data-parallel over batch; 8 cores x 2048 rows. Tiny (L, D)
weights replicated.
"""

import numpy as np

import concourse.bacc as bacc
import concourse.tile as tile
from concourse import mybir
from concourse.bass_utils import run_bass_kernel_spmd
from concourse.masks import make_identity

N_CORES = 8
B, D, L = 16384, 1024, 8
RPC = B // N_CORES          # rows per core (2048)
NT = RPC // 128             # 128-row tiles per core (16)
NCH = D // 128              # 128-wide d chunks (8)
GROUPS = [1, 3, 4, 4, 2, 2]  # tiles per group (sum = NT); small head + tail
N_WARM = 16                 # bf16 warmup matmuls to lift HAM to K=8/8

LAST_RESULTS = None


def _build(cvals):
    """Trace + compile the per-core program. cvals = [c_1..c_{L-1}]."""
    nc = bacc.Bacc("TRN2", target_bir_lowering=False, debug=False)
    f32 = mybir.dt.float32
    bf16 = mybir.dt.bfloat16

    x_d = nc.dram_tensor("x", [RPC, D], f32, kind="ExternalInput")
    wt_d = nc.dram_tensor("wt", [128, NCH * L], f32, kind="ExternalInput")
    b7_d = nc.dram_tensor("b7r", [128, D], f32, kind="ExternalInput")
    y_d = nc.dram_tensor("y", [RPC, D], f32, kind="ExternalOutput")

    # tile views: [t][p, d]
    x_tile = x_d.ap().rearrange("(t p) d -> t p d", p=128)
    x_pair = x_d.ap().rearrange("(h t p) d -> h p t d", t=2, p=128)  # 2-tile (1MB) views
    y_tile = y_d.ap().rearrange("(t p) d -> t p d", p=128)

    with tile.TileContext(nc) as tc:
        with (
            tc.tile_pool(name="const", bufs=1) as cpool,
            tc.tile_pool(name="xp", bufs=4) as xpool,
            tc.tile_pool(name="xtp", bufs=3) as xtpool,
            tc.tile_pool(name="yp", bufs=4) as ypool,
            tc.tile_pool(name="small", bufs=6) as spool,
            tc.tile_pool(name="pst", bufs=3, space="PSUM") as pst,
            tc.tile_pool(name="psu", bufs=3, space="PSUM") as psu,
            tc.tile_pool(name="psr", bufs=2, space="PSUM") as psr,
        ):
            GTMAX = max(GROUPS)

            def load_group(base_t, gt):
                """DMA gt tiles starting at tile base_t into a group tile."""
                xg = xpool.tile([128, GTMAX, D], f32, tag="xg")
                t = 0
                while t < gt:
                    if t + 2 <= gt and (base_t + t) % 2 == 0:
                        nc.sync.dma_start(
                            out=xg[:, t : t + 2, :], in_=x_pair[(base_t + t) // 2]
                        )
                        t += 2
                    else:
                        nc.sync.dma_start(out=xg[:, t, :], in_=x_tile[base_t + t])
                        t += 1
                return xg

            # --- first x data on the wire before anything else ---
            xg0 = load_group(0, GROUPS[0])

            # --- warmup: dense bf16 matmuls during initial DMA window ---
            dummy = cpool.tile([128, 512], bf16)
            nc.gpsimd.memset(dummy[:], 0.0)
            for i in range(N_WARM):
                pw = psr.tile([128, 512], f32, tag="psr")
                nc.tensor.matmul(pw[:, 0:256], dummy[:, 0:128], dummy[:, 0:256], start=True, stop=True)

            # --- constants ---
            ident = cpool.tile([128, 128], f32)
            make_identity(nc, ident[:])
            wt_sb = cpool.tile([128, NCH, L], f32)
            nc.sync.dma_start(out=wt_sb[:], in_=wt_d.ap().rearrange("p (c l) -> p c l", l=L))
            b7_sb = cpool.tile([128, D], f32)
            nc.sync.dma_start(out=b7_sb[:], in_=b7_d[:, :])
            c_sb = cpool.tile([128, L - 1], f32)
            for l in range(L - 1):
                nc.gpsimd.memset(c_sb[:, l : l + 1], cvals[l])

            base_t = 0
            for g, gt in enumerate(GROUPS):
                if g == 0:
                    xg = xg0
                else:
                    xg = load_group(base_t, gt)
                xg_c = xg[:].rearrange("p t (c d) -> p t c d", c=NCH)

                # transpose the group's chunks -> xT [128d, c, gt*128 rows]
                xT = xtpool.tile([128, NCH, GTMAX * 128], f32, tag="xT")
                for t in range(gt):
                    h = NCH // 2
                    pa = pst.tile([128, h, 128], f32, tag="pst")
                    for c in range(h):
                        nc.tensor.transpose(pa[:, c, :], xg_c[:, t, c, :], ident[:])
                    nc.scalar.copy(xT[:, 0:h, 128 * t : 128 * (t + 1)], pa[:])
                    pb = pst.tile([128, h, 128], f32, tag="pst")
                    for c in range(h):
                        nc.tensor.transpose(pb[:, c, :], xg_c[:, t, h + c, :], ident[:])
                    nc.scalar.copy(xT[:, h:NCH, 128 * t : 128 * (t + 1)], pb[:])

                # U^T for the group: [L, gt*128] = sum_c WT_c.T @ xT_c
                ps_u = psu.tile([L, GTMAX * 128], f32, tag="psu")
                for c in range(NCH):
                    nc.tensor.matmul(
                        ps_u[:, 0 : gt * 128], wt_sb[:, c, :],
                        xT[:, c, 0 : gt * 128],
                        start=(c == 0), stop=(c == NCH - 1),
                    )
                ut = spool.tile([L, GTMAX * 128], f32, tag="ut")
                nc.scalar.copy(ut[:, 0 : gt * 128], ps_u[:, 0 : gt * 128])

                for t in range(gt):
                    # U tile back to row-partition orientation: [128, L]
                    pr = psr.tile([128, L], f32, tag="psr")
                    nc.tensor.transpose(
                        pr[:], ut[:, 128 * t : 128 * (t + 1)], ident[0:L, 0:L]
                    )
                    rho0 = spool.tile([128, 1], f32, tag="rho0")
                    nc.vector.tensor_scalar_add(rho0[:], pr[:, 0:1], 1.0)
                    scano = spool.tile([128, L - 1], f32, tag="scan")
                    nc.vector.tensor_tensor_scan(
                        scano[:], pr[:, 1:L], c_sb[:], rho0[:, 0:1],
                        mybir.AluOpType.mult, mybir.AluOpType.add,
                    )
                    # out = x * rho + b7
                    yt = ypool.tile([128, D], f32)
                    nc.vector.scalar_tensor_tensor(
                        yt[:], xg[:, t, :], scano[:, L - 2 : L - 1], b7_sb[:],
                        mybir.AluOpType.mult, mybir.AluOpType.add,
                    )
                    nc.gpsimd.dma_start(out=y_tile[base_t + t], in_=yt[:])
                base_t += gt

    nc.compile()
    return nc


def kernel(x, W, b):
    global LAST_RESULTS
    x = np.ascontiguousarray(np.asarray(x), dtype=np.float32)
    W = np.ascontiguousarray(np.asarray(W), dtype=np.float32)
    b = np.ascontiguousarray(np.asarray(b), dtype=np.float32)
    assert x.shape == (B, D) and W.shape == (L, D) and b.shape == (L, D)

    cvals = [float(np.dot(b[l - 1].astype(np.float64), W[l].astype(np.float64)) + 1.0)
             for l in range(1, L)]
    wt = W.T.reshape(NCH, 128, L).transpose(1, 0, 2).reshape(128, NCH * L)
    wt = np.ascontiguousarray(wt, dtype=np.float32)
    b7r = np.ascontiguousarray(np.broadcast_to(b[L - 1], (128, D)), dtype=np.float32)

    nc = _build(cvals)

    shards = [x[i * RPC : (i + 1) * RPC] for i in range(N_CORES)]
    in_maps = [{"x": s, "wt": wt, "b7r": b7r} for s in shards]
    res = run_bass_kernel_spmd(nc, in_maps, core_ids=list(range(N_CORES)))
    LAST_RESULTS = res
    out = np.concatenate([res.results[i]["y"] for i in range(N_CORES)], axis=0)
    return out.astype(np.float32)


# revision 13
# speedup vs baseline: 1.0391x; 1.0391x over previous
"""Trainium2 Bass kernel for nn_CrossLayer (DCN-style cross stack).

Reference semantics (B=16384, D=1024, L=8):
    out_0 = x
    s_i = einsum('bd,d->b', out_i, W[i])
    out_{i+1} = x * s_i[:, None] + b[i] + x

Algebraic collapse: out_{i+1} = x * rho_{i+1} + b[i] with
    rho_1 = u_0 + 1,   rho_{l+1} = rho_l * u_l + c_l
    u_l[r] = <x[r, :], W[l]>          (U = x @ W.T, [B, L])
    c_l = <b[l-1], W[l]> + 1          (weights-only scalars)
    out = x * rho_8[:, None] + b[L-1]

Device work: U = x @ W.T (PE transposes + W-stationary matmuls), a 7-step
per-row scan, one fused scale+bias pass. x read once, out written once ->
memory-roofline bound. All arithmetic fp32 (exact vs reference).

Sharding: data-parallel over batch; 8 cores x 2048 rows. Tiny (L, D)
weights replicated.
"""

import numpy as np

import concourse.bacc as bacc
import concourse.tile as tile
from concourse import mybir
from concourse.bass_utils import run_bass_kernel_spmd
from concourse.masks import make_identity

N_CORES = 8
B, D, L = 16384, 1024, 8
RPC = B // N_CORES          # rows per core (2048)
NT = RPC // 128             # 128-row tiles per core (16)
NCH = D // 128              # 128-wide d chunks (8)
GROUPS = [4, 4, 4, 2, 2]  # tiles per group (sum = NT); small tail groups
N_WARM = 16                 # bf16 warmup matmuls to lift HAM to K=8/8

LAST_RESULTS = None


def _build(cvals):
    """Trace + compile the per-core program. cvals = [c_1..c_{L-1}]."""
    nc = bacc.Bacc("TRN2", target_bir_lowering=False, debug=False)
    f32 = mybir.dt.float32
    bf16 = mybir.dt.bfloat16

    x_d = nc.dram_tensor("x", [RPC, D], f32, kind="ExternalInput")
    wt_d = nc.dram_tensor("wt", [128, NCH * L], f32, kind="ExternalInput")
    b7_d = nc.dram_tensor("b7r", [128, D], f32, kind="ExternalInput")
    y_d = nc.dram_tensor("y", [RPC, D], f32, kind="ExternalOutput")

    # tile views: [t][p, d]
    x_tile = x_d.ap().rearrange("(t p) d -> t p d", p=128)
    x_pair = x_d.ap().rearrange("(h t p) d -> h p t d", t=2, p=128)  # 2-tile (1MB) views
    y_tile = y_d.ap().rearrange("(t p) d -> t p d", p=128)

    with tile.TileContext(nc) as tc:
        with (
            tc.tile_pool(name="const", bufs=1) as cpool,
            tc.tile_pool(name="xp", bufs=4) as xpool,
            tc.tile_pool(name="xtp", bufs=3) as xtpool,
            tc.tile_pool(name="yp", bufs=4) as ypool,
            tc.tile_pool(name="small", bufs=6) as spool,
            tc.tile_pool(name="pst", bufs=3, space="PSUM") as pst,
            tc.tile_pool(name="psu", bufs=3, space="PSUM") as psu,
            tc.tile_pool(name="psr", bufs=2, space="PSUM") as psr,
        ):
            GTMAX = max(GROUPS)

            def load_group(base_t, gt):
                """DMA gt tiles starting at tile base_t into a group tile."""
                xg = xpool.tile([128, GTMAX, D], f32, tag="xg")
                t = 0
                while t < gt:
                    if t + 2 <= gt and (base_t + t) % 2 == 0:
                        nc.sync.dma_start(
                            out=xg[:, t : t + 2, :], in_=x_pair[(base_t + t) // 2]
                        )
                        t += 2
                    else:
                        nc.sync.dma_start(out=xg[:, t, :], in_=x_tile[base_t + t])
                        t += 1
                return xg

            # --- first x data on the wire before anything else ---
            xg0 = load_group(0, GROUPS[0])

            # --- warmup: dense bf16 matmuls during initial DMA window ---
            dummy = cpool.tile([128, 512], bf16)
            nc.gpsimd.memset(dummy[:], 0.0)
            for i in range(N_WARM):
                pw = psr.tile([128, 512], f32, tag="psr")
                nc.tensor.matmul(pw[:], dummy[:, 0:128], dummy[:], start=True, stop=True)

            # --- constants ---
            ident = cpool.tile([128, 128], f32)
            make_identity(nc, ident[:])
            wt_sb = cpool.tile([128, NCH, L], f32)
            nc.sync.dma_start(out=wt_sb[:], in_=wt_d.ap().rearrange("p (c l) -> p c l", l=L))
            b7_sb = cpool.tile([128, D], f32)
            nc.sync.dma_start(out=b7_sb[:], in_=b7_d[:, :])
            c_sb = cpool.tile([128, L - 1], f32)
            for l in range(L - 1):
                nc.gpsimd.memset(c_sb[:, l : l + 1], cvals[l])

            base_t = 0
            for g, gt in enumerate(GROUPS):
                if g == 0:
                    xg = xg0
                else:
                    xg = load_group(base_t, gt)
                xg_c = xg[:].rearrange("p t (c d) -> p t c d", c=NCH)

                # transpose the group's chunks -> xT [128d, c, gt*128 rows]
                xT = xtpool.tile([128, NCH, GTMAX * 128], f32, tag="xT")
                for t in range(gt):
                    h = NCH // 2
                    pa = pst.tile([128, h, 128], f32, tag="pst")
                    for c in range(h):
                        nc.tensor.transpose(pa[:, c, :], xg_c[:, t, c, :], ident[:])
                    nc.scalar.copy(xT[:, 0:h, 128 * t : 128 * (t + 1)], pa[:])
                    pb = pst.tile([128, h, 128], f32, tag="pst")
                    for c in range(h):
                        nc.tensor.transpose(pb[:, c, :], xg_c[:, t, h + c, :], ident[:])
                    nc.scalar.copy(xT[:, h:NCH, 128 * t : 128 * (t + 1)], pb[:])

                # U^T for the group: [L, gt*128] = sum_c WT_c.T @ xT_c
                ps_u = psu.tile([L, GTMAX * 128], f32, tag="psu")
                for c in range(NCH):
                    nc.tensor.matmul(
                        ps_u[:, 0 : gt * 128], wt_sb[:, c, :],
                        xT[:, c, 0 : gt * 128],
                        start=(c == 0), stop=(c == NCH - 1),
                    )
                ut = spool.tile([L, GTMAX * 128], f32, tag="ut")
                nc.scalar.copy(ut[:, 0 : gt * 128], ps_u[:, 0 : gt * 128])

                for t in range(gt):
                    # U tile back to row-partition orientation: [128, L]
                    pr = psr.tile([128, L], f32, tag="psr")
                    nc.tensor.transpose(
                        pr[:], ut[:, 128 * t : 128 * (t + 1)], ident[0:L, 0:L]
                    )
                    rho0 = spool.tile([128, 1], f32, tag="rho0")
                    nc.vector.tensor_scalar_add(rho0[:], pr[:, 0:1], 1.0)
                    scano = spool.tile([128, L - 1], f32, tag="scan")
                    nc.vector.tensor_tensor_scan(
                        scano[:], pr[:, 1:L], c_sb[:], rho0[:, 0:1],
                        mybir.AluOpType.mult, mybir.AluOpType.add,
                    )
                    # out = x * rho + b7
                    yt = ypool.tile([128, D], f32)
                    nc.vector.scalar_tensor_tensor(
                        yt[:], xg[:, t, :], scano[:, L - 2 : L - 1], b7_sb[:],
                        mybir.AluOpType.mult, mybir.AluOpType.add,
                    )
                    nc.gpsimd.dma_start(out=y_tile[base_t + t], in_=yt[:])
                base_t += gt

    nc.compile()
    return nc


def kernel(x, W, b):
    global LAST_RESULTS
    x = np.ascontiguousarray(np.asarray(x), dtype=np.float32)
    W = np.ascontiguousarray(np.asarray(W), dtype=np.float32)
    b = np.ascontiguousarray(np.asarray(b), dtype=np.float32)
    assert x.shape == (B, D) and W.shape == (L, D) and b.shape == (L, D)

    cvals = [float(np.dot(b[l - 1].astype(np.float64), W[l].astype(np.float64)) + 1.0)
             for l in range(1, L)]
    wt = W.T.reshape(NCH, 128, L).transpose(1, 0, 2).reshape(128, NCH * L)
    wt = np.ascontiguousarray(wt, dtype=np.float32)
    b7r = np.ascontiguousarray(np.broadcast_to(b[L - 1], (128, D)), dtype=np.float32)

    nc = _build(cvals)

    shards = [x[i * RPC : (i + 1) * RPC] for i in range(N_CORES)]
    in_maps = [{"x": s, "wt": wt, "b7r": b7r} for s in shards]
    res = run_bass_kernel_spmd(nc, in_maps, core_ids=list(range(N_CORES)))
    LAST_RESULTS = res
    out = np.concatenate([res.results[i]["y"] for i in range(N_CORES)], axis=0)
    return out.astype(np.float32)
